# revision 1
# baseline (speedup 1.0000x reference)
"""Trainium2 Bass kernel for DCMLayer: 1x1 conv -> per-sample dynamic 3x3
depthwise conv -> 1x1 fuse conv, data-parallel over 8 NeuronCores.

Contract: kernel(**inputs) takes the FULL unsharded inputs
(x[32,256,96,96], conv_w[64,256], conv_b[64], dw_b[64], fuse_w[256,64],
fuse_b[256]) and returns the full y[32,256,96,96] float32.

Per-core layout: 4 samples as 2 two-sample packs. Within a pack, sample a
occupies partitions 0:64 and sample b 64:128; all matmuls touching
per-sample data use square diagonal PE tiles (64,64)@(0,0) / @(64,64).
The per-sample dynamic 3x3 depthwise conv runs on the PE as 9 diagonal
(block-diagonal content) fp16 matmuls accumulating in PSUM over a
zero-padded flat f layout (98-wide rows). g = pool(conv(x)) + b is
computed as pool(mm1 psum)/1024 + b by linearity, so x is read once.
"""
import numpy as np

import concourse.bacc as bacc
import concourse.bass as bass
import concourse.tile as tile
from concourse import mybir
from concourse.bass_utils import run_bass_kernel_spmd

F32 = mybir.dt.float32
F32R = mybir.dt.float32r
F16 = mybir.dt.float16
AF = mybir.ActivationFunctionType
ALU = mybir.AluOpType
AX = mybir.AxisListType

# Problem geometry (hardcoded per contract)
N, C, H, W = 32, 256, 96, 96
Cm, P = 64, 256
HW = H * W           # 9216
NCORES = 8
NLOC = N // NCORES   # 4 samples per core
NPACK = NLOC // 2    # 2 two-sample packs per core
KC4 = C // 64        # 4 K=64 contraction chunks for mm1
MC4 = P // 64        # 4 M=64 output chunks for mm2

WP = W + 2           # padded row width 98
FPAD = WP * (H + 2) + 2  # padded f buffer 9606 (+2 slack for corner taps)
RS = 4               # rows per compute chunk
NCH = H // RS        # 24 chunks per pack
NT = RS * W          # 384 = compute tile free size
NDW = RS * WP        # 392 dw output positions per chunk
LR = 8               # rows per x load strip
NLD = H // LR        # 12 load strips
GR = 16              # rows per y store group
NG = H // GR         # 6 store groups
BR = 32              # pooling block rows/cols

_CACHED = {}


def build_nc():
    nc = bacc.Bacc("TRN2", target_bir_lowering=False, debug=False)

    x_d = nc.dram_tensor("x", [NLOC, C, HW], F32, kind="ExternalInput").ap()
    cwB_d = nc.dram_tensor("cwB", [128, KC4 * 128], F16, kind="ExternalInput").ap()
    fwB_d = nc.dram_tensor("fwB", [128, MC4 * 128], F16, kind="ExternalInput").ap()
    cb2_d = nc.dram_tensor("cb2", [128, 1], F32, kind="ExternalInput").ap()
    fba_d = nc.dram_tensor("fba", [128, MC4], F32, kind="ExternalInput").ap()
    id_d = nc.dram_tensor("ident", [128, 128], F16, kind="ExternalInput").ap()
    y_d = nc.dram_tensor("y", [NLOC, P, HW], F32, kind="ExternalOutput").ap()

    with tile.TileContext(nc) as tc:
        build_body(nc, tc, x_d, cwB_d, fwB_d, cb2_d, fba_d, id_d, y_d)
    nc.compile()
    return nc


def build_body(nc, tc, x_d, cwB_d, fwB_d, cb2_d, fba_d, id_d, y_d):
    ctxs = []

    def pool(**kw):
        p = tc.tile_pool(**kw)
        ctxs.append(p)
        return p.__enter__()

    consts = pool(name="consts", bufs=1)
    xpool = pool(name="xs", bufs=3)
    fpads = pool(name="fpads", bufs=1)
    opool = pool(name="osb", bufs=3)
    ypool = pool(name="ysb", bufs=2)
    small = pool(name="small", bufs=1)
    diagp = pool(name="diagp", bufs=1)
    psA = pool(name="psA", bufs=2, space="PSUM")
    psD = pool(name="psD", bufs=2, space="PSUM")
    psY = pool(name="psY", bufs=1, space="PSUM")

    # ---- constants ----
    cwB = consts.tile([128, KC4 * 128], F16)  # block-diag conv_w^T chunks
    nc.sync.dma_start(cwB[:], cwB_d)
    fwB = consts.tile([128, MC4 * 128], F16)   # block-diag fuse_w^T chunks
    nc.sync.dma_start(fwB[:], fwB_d)
    cb2 = consts.tile([128, 1], F32)
    nc.sync.dma_start(cb2[:], cb2_d)
    fba = consts.tile([128, MC4], F32)
    nc.sync.dma_start(fba[:], fba_d)
    ident = consts.tile([128, 128], F16)
    nc.sync.dma_start(ident[:], id_d)

    fpad = [fpads.tile([128, FPAD], F16, tag=f"fpad{pk}", name=f"fpad{pk}")
            for pk in range(NPACK)]
    for pk in range(NPACK):
        nc.gpsimd.memset(fpad[pk][:], 0.0)

    xparts = [small.tile([128, NCH * 3], F32, tag=f"xp{pk}", name=f"xp{pk}")
              for pk in range(NPACK)]
    diag9 = [diagp.tile([128, 9 * 128], F16, tag=f"d{pk}", name=f"diag9{pk}")
             for pk in range(NPACK)]

    def phaseA_strip(pk, ld):
        sa = 2 * pk
        r0 = ld * LR
        xt = xpool.tile([128, KC4 * LR * W], F32, tag="xt", name="xt")
        for si in range(2):
            nc.sync.dma_start(
                xt[si * 64:(si + 1) * 64, :],
                x_d[sa + si].rearrange("(cc c) f -> c cc f", cc=KC4)[
                    :, :, r0 * W:(r0 + LR) * W])
        xh = xpool.tile([128, KC4 * LR * W], F16, tag="xh", name="xh")
        if ld % 2 == 0:
            nc.gpsimd.tensor_copy(xh[:], xt[:])
        else:
            nc.scalar.copy(xh[:], xt[:])
        for sub in range(LR // RS):
            ch = ld * (LR // RS) + sub
            rr = r0 + sub * RS
            pA = psA.tile([128, NT], F32, tag="pA", name="pA")
            for cc in range(KC4):
                nc.tensor.matmul(
                    pA[:],
                    cwB[:, cc * 128:(cc + 1) * 128],
                    xh[:, cc * LR * W + sub * NT:cc * LR * W + (sub + 1) * NT],
                    start=(cc == 0), stop=(cc == KC4 - 1),
                )
            # f evict: relu(psum + conv_b) -> fpad fp16, strided 98-wide rows
            base = (rr + 1) * WP + 1
            dst = fpad[pk][:, base:base + RS * WP].rearrange(
                "p (r w) -> p r w", w=WP)[:, :, 0:W]
            nc.vector.tensor_scalar(
                out=dst, in0=pA[:], scalar1=cb2[:], scalar2=0.0,
                op0=ALU.add, op1=ALU.max,
            )
            # pooling partial sums (pre-relu, pre-bias)
            pv = pA[:].rearrange("p (r cb w) -> p cb r w", r=RS, cb=3, w=BR)
            nc.vector.tensor_reduce(
                xparts[pk][:, ch * 3:(ch + 1) * 3], pv, axis=AX.XY, op=ALU.add)

    def phaseA_final(pk):
        # dynamic kernels g -> 9 diagonal fp16 weight tiles
        xp9 = small.tile([128, 9], F32, tag=f"xp9{pk}", name=f"xp9{pk}")
        nc.vector.tensor_reduce(
            xp9[:],
            xparts[pk][:].rearrange("p (br s cb) -> p br cb s",
                                    br=3, s=NCH // 3, cb=3),
            axis=AX.X, op=ALU.add)
        g = small.tile([128, 9], F32, tag=f"g{pk}", name=f"g{pk}")
        nc.vector.tensor_scalar(
            out=g[:], in0=xp9[:], scalar1=1.0 / (BR * BR), scalar2=cb2[:],
            op0=ALU.mult, op1=ALU.add)
        for t in range(9):
            nc.vector.tensor_scalar_mul(
                diag9[pk][:, t * 128:(t + 1) * 128], ident[:], g[:, t:t + 1])

    def phaseB_group(pk, gi):
        sa = 2 * pk
        ysb = ypool.tile([128, MC4 * GR * W], F32, tag="ysb", name="ysb")
        for q in range(GR // RS):
            ch = gi * (GR // RS) + q
            rr = ch * RS
            p_start = (rr + 1) * WP + 1
            pD = psD.tile([128, NDW], F32, tag="pD", name="pD")
            ti = 0
            for dy in (-1, 0, 1):
                for dx in (-1, 0, 1):
                    off = p_start + dy * WP + dx
                    nc.tensor.matmul(
                        pD[:], diag9[pk][:, ti * 128:(ti + 1) * 128],
                        fpad[pk][:, off:off + NDW],
                        start=(ti == 0), stop=(ti == 8),
                    )
                    ti += 1
            osb = opool.tile([128, NT], F16, tag="osb", name="osb")
            src = pD[:, 0:RS * WP].rearrange("p (r w) -> p r w", w=WP)[:, :, 0:W]
            nc.scalar.copy(osb[:], src)
            # mm2: one block-diag K=128 matmul per 64-channel chunk
            for mc in range(MC4):
                pY = psY.tile([128, NT], F32, tag=f"pY{mc}", name=f"pY{mc}")
                nc.tensor.matmul(
                    pY[:], fwB[:, mc * 128:(mc + 1) * 128], osb[:],
                    start=True, stop=True,
                )
                dst = ysb[:, (mc * GR + q * RS) * W:(mc * GR + q * RS) * W + NT]
                if mc % 2 == 0:
                    nc.vector.tensor_scalar_add(dst, pY[:], fba[:, mc:mc + 1])
                else:
                    nc.scalar.activation(dst, pY[:], AF.Identity,
                                         bias=fba[:, mc:mc + 1])
        return (pk, gi, ysb)

    def flush_store(item):
        pk, gi, ysb = item
        sa = 2 * pk
        for si in range(2):
            for mc in range(MC4):
                nc.sync.dma_start(
                    y_d[sa + si, mc * 64:(mc + 1) * 64,
                        gi * GR * W:(gi + 1) * GR * W],
                    ysb[si * 64:(si + 1) * 64, mc * GR * W:(mc + 1) * GR * W])

    # software pipeline: A(0); [A(pk+1) interleaved with B(pk)]; B(last)
    # stores are issued one B-group late so they never block loads in-queue
    pending = []

    def run_b(pk, gi):
        item = phaseB_group(pk, gi)
        while len(pending) > 1:
            flush_store(pending.pop(0))
        pending.append(item)

    for ld in range(NLD):
        phaseA_strip(0, ld)
    phaseA_final(0)
    for pk in range(NPACK):
        if pk + 1 < NPACK:
            for gi in range(NG):
                phaseA_strip(pk + 1, 2 * gi)
                phaseA_strip(pk + 1, 2 * gi + 1)
                run_b(pk, gi)
            phaseA_final(pk + 1)
        else:
            for gi in range(NG):
                run_b(pk, gi)
    while pending:
        flush_store(pending.pop(0))

    for p in reversed(ctxs):
        p.__exit__(None, None, None)


def _prep(inputs):
    x = np.ascontiguousarray(inputs["x"], dtype=np.float32)
    conv_w = np.asarray(inputs["conv_w"], dtype=np.float32)
    conv_b = np.asarray(inputs["conv_b"], dtype=np.float32)
    dw_b = np.asarray(inputs["dw_b"], dtype=np.float32)
    fuse_w = np.asarray(inputs["fuse_w"], dtype=np.float32)
    fuse_b = np.asarray(inputs["fuse_b"], dtype=np.float32)

    cwT = np.ascontiguousarray(conv_w.T)                      # [256, 64]
    cwB = np.zeros((128, KC4 * 128), np.float16)
    for cc in range(KC4):
        blk = cwT[cc * 64:(cc + 1) * 64, :]                   # [64 k, 64 m]
        cwB[0:64, cc * 128:cc * 128 + 64] = blk
        cwB[64:128, cc * 128 + 64:(cc + 1) * 128] = blk
    fwT = np.ascontiguousarray(fuse_w.T)                      # [64, 256]
    fwB = np.zeros((128, MC4 * 128), np.float16)
    for mc in range(MC4):
        blk = fwT[:, mc * 64:(mc + 1) * 64]                   # [64 k, 64 m]
        fwB[0:64, mc * 128:mc * 128 + 64] = blk
        fwB[64:128, mc * 128 + 64:(mc + 1) * 128] = blk
    cb2 = np.tile(conv_b, 2)[:, None].astype(np.float32)      # [128, 1]
    fba_flat = (fuse_b + fuse_w @ dw_b).astype(np.float32)    # [256]
    fba = np.stack([np.tile(fba_flat[mc * 64:(mc + 1) * 64], 2)
                    for mc in range(MC4)], axis=1)            # [128, 4]
    ident = np.eye(128, dtype=np.float16)

    xr = x.reshape(N, C, HW)
    in_maps = []
    for i in range(NCORES):
        in_maps.append({
            "x": xr[i * NLOC:(i + 1) * NLOC],
            "cwB": cwB,
            "fwB": fwB,
            "cb2": cb2,
            "fba": fba,
            "ident": ident,
        })
    return in_maps


def run(inputs, trace=False):
    if "nc" not in _CACHED:
        _CACHED["nc"] = build_nc()
    nc = _CACHED["nc"]
    in_maps = _prep(inputs)
    res = run_bass_kernel_spmd(nc, in_maps, list(range(NCORES)), trace=trace)
    y = np.concatenate([res.results[i]["y"] for i in range(NCORES)], axis=0)
    return y.reshape(N, P, H, W), res


def kernel(**inputs):
    y, _ = run(inputs, trace=False)
    return y



# revision 13
# speedup vs baseline: 1.2662x; 1.2662x over previous
"""Trainium2 Bass kernel for DCMLayer: 1x1 conv -> per-sample dynamic 3x3
depthwise conv -> 1x1 fuse conv, data-parallel over 8 NeuronCores.

Contract: kernel(**inputs) takes the FULL unsharded inputs
(x[32,256,96,96], conv_w[64,256], conv_b[64], dw_b[64], fuse_w[256,64],
fuse_b[256]) and returns the full y[32,256,96,96] float32.

v2 layout (DMA-bound fix over v1):
- x loads: one 128-partition DMA per (pack, 16-row strip), channel-major
  per sample ([c=128, s=2, cc=2, 1536] f32, 6 KB descriptor lines).
- mm1 in float32r directly on the f32 x data (no fp16 cast anywhere on
  the x path; f32r streams 1 col/cycle at N>=256). Samples a/b run as
  concurrent column-tiled matmuls (M=64 each, K=128 real channels).
- dynamic depthwise conv: 9 diagonal fp16 matmuls over a zero-haloed
  flat f16 f buffer (98-wide rows), as in v1; halo-only memsets.
- mm2 row-tiled: sample a uses PE rows 0:64, sample b rows 64:128,
  concurrent, output = 128 real channels per matmul.
- y stored as fp16 (upcast to f32 on host), one DMA per (pack, 32-row
  group): [c=128, s=2, mc=2, 3072] f16, 6 KB lines.
- loads issued on sync (SP HWDGE ring), stores on scalar (ACT ring).
"""
import numpy as np

import concourse.bacc as bacc
import concourse.bass as bass
import concourse.tile as tile
from concourse import mybir
from concourse.bass_utils import run_bass_kernel_spmd

F32 = mybir.dt.float32
F32R = mybir.dt.float32r
F16 = mybir.dt.float16
AF = mybir.ActivationFunctionType
ALU = mybir.AluOpType
AX = mybir.AxisListType

# Problem geometry (hardcoded per contract)
N, C, H, W = 32, 256, 96, 96
Cm, P = 64, 256
HW = H * W           # 9216
NCORES = 8
NLOC = N // NCORES   # 4 samples per core
NPACK = NLOC // 2    # 2 two-sample packs per core
KC4 = C // 64        # 4 K=64(x2 samples) contraction chunks for mm1
MC2 = P // 128       # 2 M=128 output chunks for mm2

WP = W + 2           # padded row width 98
FPAD = WP * (H + 2) + 2  # padded f buffer 9606 (+2 slack for corner taps)
RS = 4               # rows per compute chunk
NCH = H // RS        # 24 chunks per pack
NT = RS * W          # 384 = compute tile free size
NDW = RS * WP        # 392 dw output positions per chunk
LR = 16              # rows per x load strip
NLD = H // LR        # 6 load strips per pack
GR = 32              # rows per y store group
NG = H // GR         # 3 store groups per pack
BR = 32              # pooling block rows/cols

_CACHED = {}


def build_nc():
    nc = bacc.Bacc("TRN2", target_bir_lowering=False, debug=False)

    x_d = nc.dram_tensor("x", [NLOC, C, HW], F32R, kind="ExternalInput").ap()
    cw_d = nc.dram_tensor("cw", [128, KC4 * 128], F32R, kind="ExternalInput").ap()
    fw_d = nc.dram_tensor("fw", [128, MC2 * 128], F16, kind="ExternalInput").ap()
    cb2_d = nc.dram_tensor("cb2", [128, 1], F32, kind="ExternalInput").ap()
    fba2_d = nc.dram_tensor("fba2", [128, MC2], F32, kind="ExternalInput").ap()
    id_d = nc.dram_tensor("ident", [128, 128], F16, kind="ExternalInput").ap()
    y_d = nc.dram_tensor("y", [NLOC, P, HW], F16, kind="ExternalOutput").ap()

    with tile.TileContext(nc) as tc:
        build_body(nc, tc, x_d, cw_d, fw_d, cb2_d, fba2_d, id_d, y_d)
    nc.compile()
    return nc


def build_body(nc, tc, x_d, cw_d, fw_d, cb2_d, fba2_d, id_d, y_d):
    ctxs = []

    def pool(**kw):
        p = tc.tile_pool(**kw)
        ctxs.append(p)
        return p.__enter__()

    consts = pool(name="consts", bufs=1)
    xpool = pool(name="xs", bufs=3)
    fpads = pool(name="fpads", bufs=1)
    opool = pool(name="osb", bufs=4)
    ypool = pool(name="ysb", bufs=2)
    small = pool(name="small", bufs=1)
    diagp = pool(name="diagp", bufs=1)
    psA = pool(name="psA", bufs=2, space="PSUM")
    psD = pool(name="psD", bufs=2, space="PSUM")
    psY = pool(name="psY", bufs=2, space="PSUM")

    # ---- constants ----
    cw = consts.tile([128, KC4 * 128], F32R)   # block-diag conv_w^T chunks
    nc.sync.dma_start(cw[:], cw_d)
    fw = consts.tile([128, MC2 * 128], F16)    # fuse_w^T dup'd on both halves
    nc.sync.dma_start(fw[:], fw_d)
    cb2 = consts.tile([128, 1], F32)
    nc.sync.dma_start(cb2[:], cb2_d)
    fba2 = consts.tile([128, MC2], F32)
    nc.sync.dma_start(fba2[:], fba2_d)
    ident = consts.tile([128, 128], F16)
    nc.sync.dma_start(ident[:], id_d)

    fpad = [fpads.tile([128, FPAD], F16, tag=f"fpad{pk}", name=f"fpad{pk}")
            for pk in range(NPACK)]
    for pk in range(NPACK):
        # halo-only zeroing: top row + row0 left halo, bottom row + slack,
        # and the interleaved right|left halo column pairs
        nc.gpsimd.memset(fpad[pk][:, 0:WP + 1], 0.0)
        nc.gpsimd.memset(fpad[pk][:, (H + 1) * WP:FPAD], 0.0)
        edge = fpad[pk][:, 2 * WP - 1:2 * WP - 1 + H * WP].rearrange(
            "p (r w) -> p r w", w=WP)[:, :, 0:2]
        nc.gpsimd.memset(edge, 0.0)

    xparts = [small.tile([128, NCH * 3], F32, tag=f"xp{pk}", name=f"xp{pk}")
              for pk in range(NPACK)]
    diag9 = [diagp.tile([128, 9 * 128], F16, tag=f"d{pk}", name=f"diag9{pk}")
             for pk in range(NPACK)]

    def phaseA_strip(pk, ld):
        sa = 2 * pk
        r0 = ld * LR
        xt = xpool.tile([128, KC4 * LR * W], F32R, tag="xt", name="xt")
        xtv = xt[:].rearrange("p (cc f) -> p cc f", cc=KC4)
        for si in range(2):
            nc.sync.dma_start(
                xtv[si * 64:(si + 1) * 64],
                x_d[sa + si].rearrange("(cc c) f -> c cc f", cc=KC4)[
                    :, :, r0 * W:(r0 + LR) * W])
        for j in range(LR // RS):
            ch = ld * (LR // RS) + j
            rr = r0 + j * RS
            pA = psA.tile([128, NT], F32, tag="pA", name="pA")
            for kc in range(KC4):
                nc.tensor.matmul(
                    pA[:],
                    cw[:, kc * 128:(kc + 1) * 128],
                    xtv[:, kc:kc + 1, j * NT:(j + 1) * NT],
                    start=(kc == 0), stop=(kc == KC4 - 1),
                )
            # f evict: relu(psum + conv_b) -> fpad fp16, strided 98-wide rows
            base = (rr + 1) * WP + 1
            dst = fpad[pk][:, base:base + RS * WP].rearrange(
                "p (r w) -> p r w", w=WP)[:, :, 0:W]
            nc.vector.tensor_scalar(
                out=dst, in0=pA[:], scalar1=cb2[:], scalar2=0.0,
                op0=ALU.add, op1=ALU.max,
            )
            # pooling partial sums (pre-relu, pre-bias)
            pv = pA[:].rearrange("p (r cb w) -> p cb r w", r=RS, cb=3, w=BR)
            nc.vector.tensor_reduce(
                xparts[pk][:, ch * 3:(ch + 1) * 3], pv, axis=AX.XY, op=ALU.add)

    def phaseA_final(pk):
        # dynamic kernels g -> 9 diagonal fp16 weight tiles
        xp9 = small.tile([128, 9], F32, tag=f"xp9{pk}", name=f"xp9{pk}")
        nc.vector.tensor_reduce(
            xp9[:],
            xparts[pk][:].rearrange("p (br s cb) -> p br cb s",
                                    br=3, s=NCH // 3, cb=3),
            axis=AX.X, op=ALU.add)
        g = small.tile([128, 9], F32, tag=f"g{pk}", name=f"g{pk}")
        nc.vector.tensor_scalar(
            out=g[:], in0=xp9[:], scalar1=1.0 / (BR * BR), scalar2=cb2[:],
            op0=ALU.mult, op1=ALU.add)
        for t in range(9):
            nc.vector.tensor_scalar_mul(
                diag9[pk][:, t * 128:(t + 1) * 128], ident[:], g[:, t:t + 1])

    def phaseB_group(pk, gi):
        sa = 2 * pk
        ysb = ypool.tile([128, 2 * MC2 * GR * W], F16, tag="ysb", name="ysb")
        ysbv = ysb[:].rearrange("p (s mc f) -> p s mc f", s=2, mc=MC2)
        for q in range(GR // RS):
            ch = gi * (GR // RS) + q
            rr = ch * RS
            p_start = (rr + 1) * WP + 1
            pD = psD.tile([128, NDW], F32, tag="pD", name="pD")
            ti = 0
            for dy in (-1, 0, 1):
                for dx in (-1, 0, 1):
                    off = p_start + dy * WP + dx
                    nc.tensor.matmul(
                        pD[:], diag9[pk][:, ti * 128:(ti + 1) * 128],
                        fpad[pk][:, off:off + NDW],
                        start=(ti == 0), stop=(ti == 8),
                    )
                    ti += 1
            osb = opool.tile([128, NT], F16, tag="osb", name="osb")
            src = pD[:].rearrange("p (r w) -> p r w", w=WP)[:, :, 0:W]
            nc.scalar.copy(osb[:], src)
            # mm2: per output chunk mc, samples a/b as concurrent row tiles
            for mc in range(MC2):
                pYa = psY.tile([128, NT], F32, tag="pYa", name="pYa")
                nc.tensor.matmul(
                    pYa[:], fw[0:64, mc * 128:(mc + 1) * 128],
                    osb[0:64, :], start=True, stop=True)
                pYb = psY.tile([128, NT], F32, tag="pYb", name="pYb")
                nc.tensor.matmul(
                    pYb[:], fw[64:128, mc * 128:(mc + 1) * 128],
                    osb[64:128, :], start=True, stop=True)
                nc.vector.tensor_scalar_add(
                    ysbv[:, 0:1, mc:mc + 1, q * NT:(q + 1) * NT],
                    pYa[:], fba2[:, mc:mc + 1])
                nc.scalar.activation(
                    ysbv[:, 1:2, mc:mc + 1, q * NT:(q + 1) * NT],
                    pYb[:], AF.Identity, bias=fba2[:, mc:mc + 1])
        # store: one DMA per (pack, 32-row group), on the ACT HWDGE ring
        nc.scalar.dma_start(
            y_d[sa:sa + 2].rearrange("s (mc c) f -> c s mc f", c=128)[
                :, :, :, gi * GR * W:(gi + 1) * GR * W],
            ysbv)

    # software pipeline: A(0); [A(1) strips interleaved with B(0)]; B(1)
    for ld in range(NLD):
        phaseA_strip(0, ld)
    phaseA_final(0)
    for pk in range(NPACK):
        if pk + 1 < NPACK:
            for gi in range(NG):
                phaseA_strip(pk + 1, 2 * gi)
                phaseA_strip(pk + 1, 2 * gi + 1)
                phaseB_group(pk, gi)
            phaseA_final(pk + 1)
        else:
            for gi in range(NG):
                phaseB_group(pk, gi)

    for p in reversed(ctxs):
        p.__exit__(None, None, None)


def _prep(inputs):
    x = np.ascontiguousarray(inputs["x"], dtype=np.float32)
    conv_w = np.asarray(inputs["conv_w"], dtype=np.float32)
    conv_b = np.asarray(inputs["conv_b"], dtype=np.float32)
    dw_b = np.asarray(inputs["dw_b"], dtype=np.float32)
    fuse_w = np.asarray(inputs["fuse_w"], dtype=np.float32)
    fuse_b = np.asarray(inputs["fuse_b"], dtype=np.float32)

    cwT = np.ascontiguousarray(conv_w.T)                      # [256, 64]
    cw = np.zeros((128, KC4 * 128), np.float32)               # block-diag
    for kc in range(KC4):
        blk = cwT[kc * 64:(kc + 1) * 64, :]                   # [64 k, 64 m]
        cw[0:64, kc * 128:kc * 128 + 64] = blk
        cw[64:128, kc * 128 + 64:(kc + 1) * 128] = blk
    fwT = np.ascontiguousarray(fuse_w.T)                      # [64, 256]
    fw = np.zeros((128, MC2 * 128), np.float16)
    for mc in range(MC2):
        blk = fwT[:, mc * 128:(mc + 1) * 128]
        fw[0:64, mc * 128:(mc + 1) * 128] = blk
        fw[64:128, mc * 128:(mc + 1) * 128] = blk
    cb2 = np.tile(conv_b, 2)[:, None].astype(np.float32)      # [128, 1]
    fba_flat = (fuse_b + fuse_w @ dw_b).astype(np.float32)    # [256]
    fba2 = np.stack([fba_flat[mc * 128:(mc + 1) * 128]
                     for mc in range(MC2)], axis=1)           # [128, 2]
    ident = np.eye(128, dtype=np.float16)

    xr = x.reshape(N, C, HW)
    in_maps = []
    for i in range(NCORES):
        in_maps.append({
            "x": xr[i * NLOC:(i + 1) * NLOC],
            "cw": cw,
            "fw": fw,
            "cb2": cb2,
            "fba2": fba2,
            "ident": ident,
        })
    return in_maps


def run(inputs, trace=False):
    if "nc" not in _CACHED:
        _CACHED["nc"] = build_nc()
    nc = _CACHED["nc"]
    in_maps = _prep(inputs)
    res = run_bass_kernel_spmd(nc, in_maps, list(range(NCORES)), trace=trace)
    y = np.concatenate([res.results[i]["y"] for i in range(NCORES)], axis=0)
    return y.astype(np.float32).reshape(N, P, H, W), res


def kernel(**inputs):
    y, _ = run(inputs, trace=False)
    return y


# revision 19
# speedup vs baseline: 1.6578x; 1.3092x over previous
"""Trainium2 Bass kernel for DCMLayer: 1x1 conv -> per-sample dynamic 3x3
depthwise conv -> 1x1 fuse conv, data-parallel over 8 NeuronCores.

Contract: kernel(**inputs) takes the FULL unsharded inputs
(x[32,256,96,96], conv_w[64,256], conv_b[64], dw_b[64], fuse_w[256,64],
fuse_b[256]) and returns the full y[32,256,96,96] float32.

v2 layout (DMA-bound fix over v1):
- x loads: one 128-partition DMA per (pack, 16-row strip), channel-major
  per sample ([c=128, s=2, cc=2, 1536] f32, 6 KB descriptor lines).
- mm1 in float32r directly on the f32 x data (no fp16 cast anywhere on
  the x path; f32r streams 1 col/cycle at N>=256). Samples a/b run as
  concurrent column-tiled matmuls (M=64 each, K=128 real channels).
- dynamic depthwise conv: 9 diagonal fp16 matmuls over a zero-haloed
  flat f16 f buffer (98-wide rows), as in v1; halo-only memsets.
- mm2 row-tiled: sample a uses PE rows 0:64, sample b rows 64:128,
  concurrent, output = 128 real channels per matmul.
- y stored as fp16 (upcast to f32 on host), one DMA per (pack, 32-row
  group): [c=128, s=2, mc=2, 3072] f16, 6 KB lines.
- loads issued on sync (SP HWDGE ring), stores on scalar (ACT ring).
"""
import numpy as np

import concourse.bacc as bacc
import concourse.bass as bass
import concourse.tile as tile
from concourse import mybir
from concourse.bass_utils import run_bass_kernel_spmd

F32 = mybir.dt.float32
F32R = mybir.dt.float32r
F16 = mybir.dt.float16
AF = mybir.ActivationFunctionType
ALU = mybir.AluOpType
AX = mybir.AxisListType

# Problem geometry (hardcoded per contract)
N, C, H, W = 32, 256, 96, 96
Cm, P = 64, 256
HW = H * W           # 9216
NCORES = 8
NLOC = N // NCORES   # 4 samples per core
NPACK = NLOC // 2    # 2 two-sample packs per core
KC4 = C // 64        # 4 K=64(x2 samples) contraction chunks for mm1
MC2 = P // 128       # 2 M=128 output chunks for mm2

WP = W + 2           # padded row width 98
FPAD = WP * (H + 2) + 2  # padded f buffer 9606 (+2 slack for corner taps)
RS = 4               # rows per compute chunk
NCH = H // RS        # 24 chunks per pack
NT = RS * W          # 384 = compute tile free size
NDW = RS * WP        # 392 dw output positions per chunk
LR = 32              # rows per x load strip
NLD = H // LR        # 3 load strips per pack
GR = 48              # rows per y store group
NG = H // GR         # 2 store groups per pack
BR = 32              # pooling block rows/cols

_CACHED = {}


def build_nc():
    nc = bacc.Bacc("TRN2", target_bir_lowering=False, debug=False)

    x_d = nc.dram_tensor("x", [NLOC, C, HW], F32, kind="ExternalInput").ap()
    cw_d = nc.dram_tensor("cw", [128, KC4 * 128], F16, kind="ExternalInput").ap()
    fw_d = nc.dram_tensor("fw", [128, MC2 * 128], F16, kind="ExternalInput").ap()
    cb2_d = nc.dram_tensor("cb2", [128, 1], F32, kind="ExternalInput").ap()
    fba2_d = nc.dram_tensor("fba2", [128, MC2], F32, kind="ExternalInput").ap()
    id_d = nc.dram_tensor("ident", [128, 128], F16, kind="ExternalInput").ap()
    y_d = nc.dram_tensor("y", [NLOC, P, HW], F16, kind="ExternalOutput").ap()

    with tile.TileContext(nc) as tc:
        build_body(nc, tc, x_d, cw_d, fw_d, cb2_d, fba2_d, id_d, y_d)
    nc.compile()
    return nc


def build_body(nc, tc, x_d, cw_d, fw_d, cb2_d, fba2_d, id_d, y_d):
    ctxs = []

    def pool(**kw):
        p = tc.tile_pool(**kw)
        ctxs.append(p)
        return p.__enter__()

    consts = pool(name="consts", bufs=1)
    xpool = pool(name="xs", bufs=3)
    fpads = pool(name="fpads", bufs=1)
    opool = pool(name="osb", bufs=4)
    ypool = pool(name="ysb", bufs=2)
    small = pool(name="small", bufs=1)
    diagp = pool(name="diagp", bufs=1)
    psA = pool(name="psA", bufs=2, space="PSUM")
    psD = pool(name="psD", bufs=2, space="PSUM")
    psY = pool(name="psY", bufs=2, space="PSUM")

    # ---- constants ----
    cw = consts.tile([128, KC4 * 128], F16)    # block-diag conv_w^T chunks
    nc.sync.dma_start(cw[:], cw_d)
    fw = consts.tile([128, MC2 * 128], F16)    # fuse_w^T dup'd on both halves
    nc.sync.dma_start(fw[:], fw_d)
    cb2 = consts.tile([128, 1], F32)
    nc.sync.dma_start(cb2[:], cb2_d)
    fba2 = consts.tile([128, MC2], F32)
    nc.sync.dma_start(fba2[:], fba2_d)
    ident = consts.tile([128, 128], F16)
    nc.sync.dma_start(ident[:], id_d)

    fpad = [fpads.tile([128, FPAD], F16, tag=f"fpad{pk}", name=f"fpad{pk}")
            for pk in range(NPACK)]
    for pk in range(NPACK):
        # halo-only zeroing: top row + row0 left halo, bottom row + slack,
        # and the interleaved right|left halo column pairs
        nc.gpsimd.memset(fpad[pk][:, 0:WP + 1], 0.0)
        nc.gpsimd.memset(fpad[pk][:, (H + 1) * WP:FPAD], 0.0)
        edge = fpad[pk][:, 2 * WP - 1:2 * WP - 1 + H * WP].rearrange(
            "p (r w) -> p r w", w=WP)[:, :, 0:2]
        nc.gpsimd.memset(edge, 0.0)

    xparts = [small.tile([128, NCH * 3], F32, tag=f"xp{pk}", name=f"xp{pk}")
              for pk in range(NPACK)]
    diag9 = [diagp.tile([128, 9 * 128], F16, tag=f"d{pk}", name=f"diag9{pk}")
             for pk in range(NPACK)]

    def phaseA_strip(pk, ld):
        sa = 2 * pk
        r0 = ld * LR
        xt = xpool.tile([128, KC4 * LR * W], F16, tag="xt", name="xt")
        xtv = xt[:].rearrange("p (cc f) -> p cc f", cc=KC4)
        for si in range(2):
            # SWDGE cast-DMA: reads f32 x from HBM, writes fp16 into SBUF
            nc.gpsimd.dma_start(
                xtv[si * 64:(si + 1) * 64],
                x_d[sa + si].rearrange("(cc c) f -> c cc f", cc=KC4)[
                    :, :, r0 * W:(r0 + LR) * W])
        for j in range(LR // RS):
            ch = ld * (LR // RS) + j
            rr = r0 + j * RS
            pA = psA.tile([128, NT], F32, tag="pA", name="pA")
            for kc in range(KC4):
                nc.tensor.matmul(
                    pA[:],
                    cw[:, kc * 128:(kc + 1) * 128],
                    xtv[:, kc:kc + 1, j * NT:(j + 1) * NT],
                    start=(kc == 0), stop=(kc == KC4 - 1),
                )
            # f evict: relu(psum + conv_b) -> fpad fp16, strided 98-wide rows
            base = (rr + 1) * WP + 1
            dst = fpad[pk][:, base:base + RS * WP].rearrange(
                "p (r w) -> p r w", w=WP)[:, :, 0:W]
            nc.vector.tensor_scalar(
                out=dst, in0=pA[:], scalar1=cb2[:], scalar2=0.0,
                op0=ALU.add, op1=ALU.max,
            )
            # pooling partial sums (pre-relu, pre-bias)
            pv = pA[:].rearrange("p (r cb w) -> p cb r w", r=RS, cb=3, w=BR)
            nc.vector.tensor_reduce(
                xparts[pk][:, ch * 3:(ch + 1) * 3], pv, axis=AX.XY, op=ALU.add)

    def phaseA_final(pk):
        # dynamic kernels g -> 9 diagonal fp16 weight tiles
        xp9 = small.tile([128, 9], F32, tag=f"xp9{pk}", name=f"xp9{pk}")
        nc.vector.tensor_reduce(
            xp9[:],
            xparts[pk][:].rearrange("p (br s cb) -> p br cb s",
                                    br=3, s=NCH // 3, cb=3),
            axis=AX.X, op=ALU.add)
        g = small.tile([128, 9], F32, tag=f"g{pk}", name=f"g{pk}")
        nc.vector.tensor_scalar(
            out=g[:], in0=xp9[:], scalar1=1.0 / (BR * BR), scalar2=cb2[:],
            op0=ALU.mult, op1=ALU.add)
        for t in range(9):
            nc.vector.tensor_scalar_mul(
                diag9[pk][:, t * 128:(t + 1) * 128], ident[:], g[:, t:t + 1])

    def phaseB_group(pk, gi):
        sa = 2 * pk
        ysb = ypool.tile([128, 2 * MC2 * GR * W], F16, tag="ysb", name="ysb")
        ysbv = ysb[:].rearrange("p (s mc f) -> p s mc f", s=2, mc=MC2)
        for q in range(GR // RS):
            ch = gi * (GR // RS) + q
            rr = ch * RS
            p_start = (rr + 1) * WP + 1
            pD = psD.tile([128, NDW], F32, tag="pD", name="pD")
            ti = 0
            for dy in (-1, 0, 1):
                for dx in (-1, 0, 1):
                    off = p_start + dy * WP + dx
                    nc.tensor.matmul(
                        pD[:], diag9[pk][:, ti * 128:(ti + 1) * 128],
                        fpad[pk][:, off:off + NDW],
                        start=(ti == 0), stop=(ti == 8),
                    )
                    ti += 1
            osb = opool.tile([128, NT], F16, tag="osb", name="osb")
            src = pD[:].rearrange("p (r w) -> p r w", w=WP)[:, :, 0:W]
            nc.scalar.copy(osb[:], src)
            # mm2: per output chunk mc, samples a/b as concurrent row tiles
            for mc in range(MC2):
                pYa = psY.tile([128, NT], F32, tag="pYa", name="pYa")
                nc.tensor.matmul(
                    pYa[:], fw[0:64, mc * 128:(mc + 1) * 128],
                    osb[0:64, :], start=True, stop=True)
                pYb = psY.tile([128, NT], F32, tag="pYb", name="pYb")
                nc.tensor.matmul(
                    pYb[:], fw[64:128, mc * 128:(mc + 1) * 128],
                    osb[64:128, :], start=True, stop=True)
                nc.vector.tensor_scalar_add(
                    ysbv[:, 0:1, mc:mc + 1, q * NT:(q + 1) * NT],
                    pYa[:], fba2[:, mc:mc + 1])
                nc.scalar.activation(
                    ysbv[:, 1:2, mc:mc + 1, q * NT:(q + 1) * NT],
                    pYb[:], AF.Identity, bias=fba2[:, mc:mc + 1])
        # store: one DMA per (pack, 32-row group), on the ACT HWDGE ring
        nc.scalar.dma_start(
            y_d[sa:sa + 2].rearrange("s (mc c) f -> c s mc f", c=128)[
                :, :, :, gi * GR * W:(gi + 1) * GR * W],
            ysbv)

    # software pipeline: A(0); [A(1) strips interleaved with B(0)]; B(1)
    # NLD strips spread across NG store-groups of the previous pack
    strip_sched = [[], []]
    for ld in range(NLD):
        strip_sched[min(ld * NG // NLD, NG - 1)].append(ld)
    for ld in range(NLD):
        phaseA_strip(0, ld)
    phaseA_final(0)
    for pk in range(NPACK):
        if pk + 1 < NPACK:
            for gi in range(NG):
                for ld in strip_sched[gi]:
                    phaseA_strip(pk + 1, ld)
                phaseB_group(pk, gi)
            phaseA_final(pk + 1)
        else:
            for gi in range(NG):
                phaseB_group(pk, gi)

    for p in reversed(ctxs):
        p.__exit__(None, None, None)


def _prep(inputs):
    x = np.ascontiguousarray(inputs["x"], dtype=np.float32)
    conv_w = np.asarray(inputs["conv_w"], dtype=np.float32)
    conv_b = np.asarray(inputs["conv_b"], dtype=np.float32)
    dw_b = np.asarray(inputs["dw_b"], dtype=np.float32)
    fuse_w = np.asarray(inputs["fuse_w"], dtype=np.float32)
    fuse_b = np.asarray(inputs["fuse_b"], dtype=np.float32)

    cwT = np.ascontiguousarray(conv_w.T)                      # [256, 64]
    cw = np.zeros((128, KC4 * 128), np.float16)               # block-diag
    for kc in range(KC4):
        blk = cwT[kc * 64:(kc + 1) * 64, :]                   # [64 k, 64 m]
        cw[0:64, kc * 128:kc * 128 + 64] = blk
        cw[64:128, kc * 128 + 64:(kc + 1) * 128] = blk
    fwT = np.ascontiguousarray(fuse_w.T)                      # [64, 256]
    fw = np.zeros((128, MC2 * 128), np.float16)
    for mc in range(MC2):
        blk = fwT[:, mc * 128:(mc + 1) * 128]
        fw[0:64, mc * 128:(mc + 1) * 128] = blk
        fw[64:128, mc * 128:(mc + 1) * 128] = blk
    cb2 = np.tile(conv_b, 2)[:, None].astype(np.float32)      # [128, 1]
    fba_flat = (fuse_b + fuse_w @ dw_b).astype(np.float32)    # [256]
    fba2 = np.stack([fba_flat[mc * 128:(mc + 1) * 128]
                     for mc in range(MC2)], axis=1)           # [128, 2]
    ident = np.eye(128, dtype=np.float16)

    xr = x.reshape(N, C, HW)
    in_maps = []
    for i in range(NCORES):
        in_maps.append({
            "x": xr[i * NLOC:(i + 1) * NLOC],
            "cw": cw,
            "fw": fw,
            "cb2": cb2,
            "fba2": fba2,
            "ident": ident,
        })
    return in_maps


def run(inputs, trace=False):
    if "nc" not in _CACHED:
        _CACHED["nc"] = build_nc()
    nc = _CACHED["nc"]
    in_maps = _prep(inputs)
    res = run_bass_kernel_spmd(nc, in_maps, list(range(NCORES)), trace=trace)
    y = np.concatenate([res.results[i]["y"] for i in range(NCORES)], axis=0)
    return y.astype(np.float32).reshape(N, P, H, W), res


def kernel(**inputs):
    y, _ = run(inputs, trace=False)
    return y
